# revision 1
# baseline (speedup 1.0000x reference)
"""FKLoss TRN2 kernel v7: UR5 FK euler+translation L1 loss, data-parallel 8 cores.

v7 vs v6 (54.6us):
 - hx == sy identity (hx^2 = u^2+c234^2 = 1-R20n^2 = w5^2+c5^2): the whole
   hx branch (u^2, c234^2, hx2-add, half the sqrt) is deleted
 - cos of the critical group (t5, a234) via ARW(shift=pi/2) on DVE, so
   COS-A does not wait for an ACT Abs; group B keeps the Abs path
 - explicit order-only deps: t2w/t1w after the critical wrap chain,
   translation-delta after the last QT mul, ABS-half1 after arctan-y
"""
import numpy as np
import concourse.bass as bass
import concourse.tile as tile
from concourse import bacc, mybir
from concourse.bass_utils import run_bass_kernel_spmd

A = mybir.ActivationFunctionType
OP = mybir.AluOpType
F32 = mybir.dt.float32
BF16 = mybir.dt.bfloat16

PI = float(np.pi)
B = 524288
NCORES = 8
BC = B // NCORES
P = 128
F = BC // P
W = 6 * F

A2, A3 = -0.425, -0.39225
D1, D4, D5, D6 = 0.089159, 0.10915, 0.09465, 0.0823
EPS = 1e-30


def _dep(later, earlier):
    bass._add_dep_helper(later.ins, earlier.ins, sync=False, reason="sched-order")


def build():
    nc = bacc.Bacc("TRN2", target_bir_lowering=False, debug=False, num_devices=NCORES)

    pred_d = nc.dram_tensor("pred", [P, W], F32, kind="ExternalInput")
    gt_d = nc.dram_tensor("gt", [P, W], F32, kind="ExternalInput")
    ph_d = nc.dram_tensor("pihalf", [P, 1], F32, kind="ExternalInput")
    out_d = nc.dram_tensor("out", [P, 2], F32, kind="ExternalOutput")

    with tile.TileContext(nc) as tc:
        with tc.tile_pool(name="pool", bufs=1) as pool:
            def mk(name, cols=F, dt=F32):
                return pool.tile([P, cols], dt, tag=name, name=name)

            ph = pool.tile([P, 1], F32, tag="ph", name="ph")
            nc.sync.dma_start(ph[:], ph_d.ap())
            inP = mk("inP", W)
            nc.sync.dma_start(inP[:], pred_d.ap())
            inG = mk("inG", W)
            with tc.tile_wait_until(0.016):
                nc.sync.dma_start(inG[:], gt_d.ap())

            pc3 = inP[:].rearrange("p (f c) -> p c f", c=6)
            gc3 = inG[:].rearrange("p (f c) -> p c f", c=6)
            th = [pc3[:, j, :] for j in range(6)]

            V, S = nc.vector, nc.scalar

            def arw(out, in_, shift):
                return V.add_range_wrap(out=out, in_=in_, shift=shift, bound=PI, period=2 * PI)

            # --- TH = [t5w | a234w | a23w | t2w | t1w]; THCA = [t5c | a234c] ---
            TH = mk("TH", 5 * F)
            THCA = mk("THCA", 2 * F)
            arw(TH[:, 0 * F:1 * F], th[4], 0.0)            # t5w
            a23r = mk("a23r")
            V.tensor_tensor(out=a23r[:], in0=th[1], in1=th[2], op=OP.add)
            a234r = mk("a234r")
            V.tensor_tensor(out=a234r[:], in0=a23r[:], in1=th[3], op=OP.add)
            arw(TH[:, 1 * F:2 * F], a234r[:], 0.0)         # a234w
            arw(THCA[:, 0:F], th[4], PI / 2)               # t5c input
            ca = arw(THCA[:, F:2 * F], a234r[:], PI / 2)   # a234c input
            w_a23 = arw(TH[:, 2 * F:3 * F], a23r[:], 0.0)  # a23w
            w_t2 = arw(TH[:, 3 * F:4 * F], th[1], 0.0)     # t2w
            w_t1 = arw(TH[:, 4 * F:5 * F], th[0], 0.0)     # t1w
            _dep(w_a23, ca)
            _dep(w_t2, ca)
            _dep(w_t1, ca)

            # --- trig: group A (chunks 0-1) cos via THCA; group B via Abs ---
            SC = mk("SC", 10 * F, BF16)
            ABT = mk("ABT", 3 * F)
            S.activation(SC[:, 0:2 * F], TH[:, 0:2 * F], A.Sin)
            S.activation(SC[:, 5 * F:7 * F], THCA[:], A.Sin)
            S.activation(SC[:, 2 * F:5 * F], TH[:, 2 * F:5 * F], A.Sin)
            S.activation(ABT[:], TH[:, 2 * F:5 * F], A.Abs)
            S.activation(SC[:, 7 * F:10 * F], ABT[:], A.Sin, scale=-1.0, bias=ph[:])
            s5 = SC[:, 0 * F:1 * F]; s234 = SC[:, 1 * F:2 * F]; s23 = SC[:, 2 * F:3 * F]
            s2 = SC[:, 3 * F:4 * F]; s1 = SC[:, 4 * F:5 * F]
            c5 = SC[:, 5 * F:6 * F]; c234 = SC[:, 6 * F:7 * F]; c23 = SC[:, 7 * F:8 * F]
            c2 = SC[:, 8 * F:9 * F]; c1 = SC[:, 9 * F:10 * F]

            # --- products UWR = [u | w5 | R20n] (bf16 2x) ---
            UWR = mk("UWR", 3 * F, BF16)
            u = UWR[:, 0:F]; w5 = UWR[:, F:2 * F]; R20n = UWR[:, 2 * F:3 * F]
            V.tensor_tensor(out=u, in0=s234, in1=c5, op=OP.mult)
            V.tensor_tensor(out=w5, in0=c234, in1=s5, op=OP.mult)
            V.tensor_tensor(out=R20n, in0=s234, in1=s5, op=OP.mult)

            # --- sy = sqrt(w5^2 + c5^2)  (also equals hx) ---
            W5q = mk("W5q")
            S.activation(W5q[:], w5, A.Square)
            C5q = mk("C5q")
            S.activation(C5q[:], c5, A.Square)
            HS = mk("HS")
            V.tensor_tensor(out=HS[:], in0=W5q[:], in1=C5q[:], op=OP.add)
            SQ = mk("SQ", F, BF16)
            S.activation(SQ[:], HS[:], A.Sqrt)

            # --- gt deinterleave on ACT (fills the pre-arctan idle window) ---
            GD = mk("GD", 6 * F)
            S.activation(GD[:, 0:3 * F].rearrange("p (c f) -> p c f", f=F),
                         gc3[:, 0:3, :], A.Copy)
            S.activation(GD[:, 3 * F:6 * F].rearrange("p (c f) -> p c f", f=F),
                         gc3[:, 3:6, :], A.Copy)

            # --- dens: denx = sy+u, denz = sy+w5 (bf16), clamps, recip ---
            DEN = mk("DEN", 2 * F, BF16)
            V.tensor_tensor(out=DEN[:, 0:F], in0=SQ[:], in1=u, op=OP.add)
            V.tensor_tensor(out=DEN[:, F:2 * F], in0=SQ[:], in1=w5, op=OP.add)
            RC = mk("RC", 3 * F)
            V.tensor_scalar_max(RC[:, 0:2 * F], DEN[:], EPS)
            V.tensor_scalar_max(RC[:, 2 * F:3 * F], SQ[:], EPS)
            REC = mk("REC", 3 * F)
            V.reciprocal_approx_fast(out=REC[:], in_=RC[:])

            # --- QT = [qx | qz | qy]; atans; XYZ = [x | y | z] ---
            QT = mk("QT", 3 * F)
            V.tensor_tensor(out=QT[:, 0:F], in0=REC[:, 0:F], in1=c234, op=OP.mult)
            V.tensor_tensor(out=QT[:, F:2 * F], in0=REC[:, F:2 * F], in1=c5, op=OP.mult)
            qlast = V.tensor_tensor(out=QT[:, 2 * F:3 * F], in0=REC[:, 2 * F:3 * F], in1=R20n, op=OP.mult)
            AT = mk("AT", 2 * F)
            XYZ = mk("XYZ", 3 * F)
            S.activation(AT[:], QT[:, 0:2 * F], A.Arctan)                 # [atx | atz]
            aty_i = S.activation(XYZ[:, F:2 * F], QT[:, 2 * F:3 * F], A.Arctan)  # y
            atx = AT[:, 0:F]; atz = AT[:, F:2 * F]

            zp = mk("zp")
            V.scalar_tensor_tensor(out=zp[:], in0=atz, scalar=2.0, in1=TH[:, 4 * F:5 * F],
                                   op0=OP.mult, op1=OP.add)
            xp = mk("xp")
            V.scalar_tensor_tensor(out=xp[:], in0=atx, scalar=-2.0, in1=th[5],
                                   op0=OP.mult, op1=OP.add)
            arw(XYZ[:, 0:F], xp[:], -PI / 2)       # x
            arw(XYZ[:, 2 * F:3 * F], zp[:], 0.0)   # z

            # --- translation (negated, bf16): TRN = [-tx | -ty | -tz] ---
            pcA = mk("pcA", F, BF16)
            V.tensor_scalar_mul(pcA[:], c23, A3)
            pcw = mk("pcw", F, BF16)
            V.scalar_tensor_tensor(out=pcw[:], in0=c2, scalar=A2, in1=pcA[:], op0=OP.mult, op1=OP.add)
            pcd = mk("pcd", F, BF16)
            V.scalar_tensor_tensor(out=pcd[:], in0=s234, scalar=D5, in1=pcw[:], op0=OP.mult, op1=OP.add)
            wa = mk("wa", F, BF16)
            V.scalar_tensor_tensor(out=wa[:], in0=w5, scalar=-D6, in1=pcd[:], op0=OP.mult, op1=OP.add)
            wb = mk("wb", F, BF16)
            V.tensor_scalar(wb[:], c5, D6, D4, op0=OP.mult, op1=OP.add)
            m1 = mk("m1", F, BF16); m2 = mk("m2", F, BF16)
            V.tensor_tensor(out=m1[:], in0=c1, in1=wa[:], op=OP.mult)
            V.tensor_tensor(out=m2[:], in0=s1, in1=wb[:], op=OP.mult)
            TRN = mk("TRN", 3 * F, BF16)
            V.tensor_tensor(out=TRN[:, 0:F], in0=m1[:], in1=m2[:], op=OP.add)       # -tx
            m3 = mk("m3", F, BF16); m4 = mk("m4", F, BF16)
            V.tensor_tensor(out=m3[:], in0=s1, in1=wa[:], op=OP.mult)
            V.tensor_tensor(out=m4[:], in0=c1, in1=wb[:], op=OP.mult)
            V.tensor_tensor(out=TRN[:, F:2 * F], in0=m3[:], in1=m4[:], op=OP.subtract)  # -ty
            z1t = mk("z1t", F, BF16)
            V.tensor_scalar(z1t[:], s2, -A2, -D1, op0=OP.mult, op1=OP.add)
            z2t = mk("z2t", F, BF16)
            V.scalar_tensor_tensor(out=z2t[:], in0=s23, scalar=-A3, in1=z1t[:], op0=OP.mult, op1=OP.add)
            z3t = mk("z3t", F, BF16)
            V.scalar_tensor_tensor(out=z3t[:], in0=c234, scalar=D5, in1=z2t[:], op0=OP.mult, op1=OP.add)
            V.scalar_tensor_tensor(out=TRN[:, 2 * F:3 * F], in0=R20n, scalar=D6, in1=z3t[:],
                                   op0=OP.mult, op1=OP.add)                          # -tz

            # --- deltas: translations (held after QT), angles last ---
            DT = mk("DT", 6 * F)
            DS = mk("DS", 6 * F)
            acc = pool.tile([P, 2], F32, tag="acc", name="acc")
            td = V.tensor_tensor(out=DT[:, 0:3 * F], in0=TRN[:], in1=GD[:, 3 * F:6 * F], op=OP.add)
            _dep(td, qlast)
            ab1 = S.activation(DS[:, 0:3 * F], DT[:, 0:3 * F], A.Abs, accum_out=acc[:, 0:1])
            _dep(ab1, aty_i)
            V.tensor_tensor(out=DT[:, 3 * F:6 * F], in0=XYZ[:], in1=GD[:, 0:3 * F], op=OP.subtract)
            S.activation(DS[:, 3 * F:6 * F], DT[:, 3 * F:6 * F], A.Abs, accum_out=acc[:, 1:2])

            nc.sync.dma_start(out_d.ap(), acc[:])

    nc.compile()
    return nc


_NC = None


def get_nc():
    global _NC
    if _NC is None:
        _NC = build()
    return _NC


_PIHALF = np.full((P, 1), PI / 2, dtype=np.float32)


def make_in_maps(pred, gt):
    in_maps = []
    for i in range(NCORES):
        sl = slice(i * BC, (i + 1) * BC)
        in_maps.append({
            "pred": pred[sl].reshape(P, W),
            "gt": gt[sl].reshape(P, W),
            "pihalf": _PIHALF,
        })
    return in_maps


def kernel(pred_ja_batch: np.ndarray, gt_eatv_batch: np.ndarray) -> np.ndarray:
    nc = get_nc()
    pred = np.ascontiguousarray(pred_ja_batch, dtype=np.float32)
    gt = np.ascontiguousarray(gt_eatv_batch, dtype=np.float32)
    res = run_bass_kernel_spmd(nc, make_in_maps(pred, gt), core_ids=list(range(NCORES)))
    total = np.float64(0.0)
    for r in res.results:
        total += r["out"].astype(np.float64).sum()
    return np.float32(total / (B * 6))
